# revision 38
# baseline (speedup 1.0000x reference)
"""Trainium2 Bass kernel for EuclideanCodebook (VQ) eval-mode forward.

Full-input contract: kernel(x, embed) -> (quantize, embed_ind, residual)

Data-parallel over 8 NeuronCores (8192 tokens/core), codebook replicated.
Per 128-token tile the device computes scores h = (2x)·e^T - |e|^2 with
fp32 PE matmuls (argmax h == argmin dist, fp32-exact), finds the argmax
with DVE max/max_index, gathers the winning codebook rows with batched
indirect DMAs, and produces the residual as quantize + (-x) via a
DMA-engine accumulate (exact fp32, identical rounding to quantize - x).

The walrus build in this toolchain accepts at most ONE sync wait per
instruction, so the kernel keeps every instruction at <=1 cross wait:
SBUF rings spanning the shard (no slot reuse), real absorber instructions
that pre-advance engine clocks, SWDGE (gpsimd) output DMAs whose producers
collapse to a single semaphore, and a DRAM->DRAM accumulate whose source
is a never-written input tensor (zero producer deps).
"""

import os
import sys

for _p in ("/opt/trn_rl_repo", os.path.expanduser("~/.axon_site/_ro/trn_rl_repo")):
    if os.path.isdir(_p) and _p not in sys.path:
        sys.path.insert(0, _p)

import numpy as np

import concourse.bass as bass
import concourse.mybir as mybir
import concourse.tile as tile
from concourse.tile_rust import add_dep_helper

N_CORES = 8
DIM = 256
K = 1024
N_TOK = 32 * 2048
TOK_PER_CORE = N_TOK // N_CORES
P = 128
FP32 = mybir.dt.float32
U32 = mybir.dt.uint32
BF16 = mybir.dt.bfloat16

GATHER_TILES = 8  # tiles per indirect-DMA gather call
IOUT_GROUPS = 2  # gather groups per embed_ind output DMA


def build_bass(n_tok_core: int = TOK_PER_CORE) -> bass.Bass:
    n_tiles = n_tok_core // P
    g = GATHER_TILES
    assert n_tiles % (g * IOUT_GROUPS) == 0
    nc = bass.Bass()
    xt2_in = nc.declare_dram_parameter("xt2", [n_tiles, 2, P, P], FP32, isOutput=False)
    xneg_in = nc.declare_dram_parameter("xneg", [n_tok_core, DIM], FP32, isOutput=False)
    e_in = nc.declare_dram_parameter("embed", [K, DIM], FP32, isOutput=False)
    et_in = nc.declare_dram_parameter("embedT", [2, P, K], FP32, isOutput=False)
    ne2_in = nc.declare_dram_parameter("nege2", [P, K], FP32, isOutput=False)

    q_out = nc.declare_dram_parameter("quantize", [n_tok_core, DIM], FP32, isOutput=True)
    i_out = nc.declare_dram_parameter("embed_ind", [n_tok_core, 1], U32, isOutput=True)
    r_out = nc.declare_dram_parameter("residual", [n_tok_core, DIM], FP32, isOutput=True)

    with tile.TileContext(nc) as tc:
        with tc.tile_pool(name="const", bufs=1) as cpool:
            eT = [
                cpool.tile([P, K], FP32, tag=f"eT_{h}", name=f"eT_{h}")
                for h in range(2)
            ]
            nege2 = cpool.tile([P, K], FP32, tag="nege2")
            # Rings spanning the whole shard: no slot reuse anywhere.
            xT_all = cpool.tile([P, n_tiles * DIM], FP32, tag="xT_all")
            # one gather-output tile per group: the indirect DMA's write is
            # tracked whole-tensor, so sharing one ring would serialize groups
            qt_g = [
                cpool.tile([P, GATHER_TILES * DIM], FP32, tag=f"qt_{c}",
                           name=f"qt_{c}")
                for c in range(min(4, n_tiles // GATHER_TILES))
            ]
            xneg_all = cpool.tile([P, n_tiles * DIM], FP32, tag="xneg_all")
            idx_all = cpool.tile([P, n_tiles * 8], U32, tag="idx_all")
            idxp_all = cpool.tile([P, n_tiles], U32, tag="idxp_all")
            mx_all = cpool.tile([P, 16 * 8], FP32, tag="mx_all")
            touch = cpool.tile([P, 2 * n_tiles], FP32, tag="touch")
            stouch = cpool.tile([P, 48], FP32, tag="stouch")

            for h in range(2):
                nc.sync.dma_start(out=eT[h][:], in_=et_in[h])
            nc.sync.dma_start(out=nege2[:], in_=ne2_in[:])

            with (
                tc.tile_pool(name="hp", bufs=1) as hp,
                tc.tile_pool(name="psh", bufs=3, space="PSUM") as psh,
            ):
                # absorb const-load ticks on their consumer engines
                for h in range(2):
                    nc.tensor.ldweights(weights=eT[h][:, :8].bitcast(BF16))
                nc.vector.tensor_copy(touch[:, :2], nege2[:, :2])

                add_insts = {}
                maxi_insts = {}
                qout_insts = {}
                swdge_log = []
                hwdge_log = []
                pe_log = []
                dve_log = []
                pool_log = []

                def swdge(emit_fn):
                    # SWDGE sem-slot reuse (lap >= 2 of the 8-sem round robin)
                    # makes Tile add a ring-credit wait; pre-absorb it on a
                    # tiny Pool instruction so the DMA keeps <=1 wait.
                    n = len(swdge_log)
                    for k in range(max(0, n - 8), n):
                        ab = nc.gpsimd.memset(
                            stouch[:1, k % 16 : k % 16 + 1], 0.0
                        )
                        pool_log.append(ab)
                        add_dep_helper(
                            ab.ins, swdge_log[k].ins,
                            reason="Pool absorbs SWDGE ring tick",
                        )
                    inst = emit_fn()
                    swdge_log.append(inst)
                    return inst

                def logged(log, inst):
                    log.append(inst)
                    return inst

                xneg_dmas = {}
                for i in range(n_tiles):
                    csl = slice(i * DIM, (i + 1) * DIM)

                    hwdge_log.append(nc.sync.dma_start(
                        out=xT_all[:, csl].rearrange("p (h t) -> p h t", h=2),
                        in_=xt2_in[i].rearrange("h d t -> d h t"),
                    ))
                    xneg_dmas[i] = nc.sync.dma_start(
                        out=xneg_all[:, csl], in_=xneg_in[i * P : (i + 1) * P, :]
                    )
                    hwdge_log.append(xneg_dmas[i])
                    # PE absorbers: the xT DMA tick, then the DVE tick that
                    # freed this tile's PSUM slot.
                    ldw = nc.tensor.ldweights(
                        weights=xT_all[:, i * DIM : i * DIM + 8].bitcast(BF16)
                    )
                    pe_abs = None
                    if i - 3 >= 0:
                        pe_abs = nc.tensor.ldweights(weights=eT[0][:, :8].bitcast(BF16))
                        add_dep_helper(
                            pe_abs.ins,
                            add_insts[i - 3].ins,
                            reason="PE absorbs PSUM-release tick",
                        )

                    hps = psh.tile([P, K], FP32, tag="hps", space="PSUM")
                    first = True
                    for half in range(2):
                        sl = slice(half * 512, (half + 1) * 512)
                        for h in range(2):
                            mm = nc.tensor.matmul(
                                hps[:, sl],
                                lhsT=xT_all[:, i * DIM + h * P : i * DIM + (h + 1) * P],
                                rhs=eT[h][:, sl],
                                start=(h == 0),
                                stop=(h == 1),
                            )
                            pe_log.append(mm)
                            if first:
                                first = False
                                add_dep_helper(mm.ins, ldw.ins, sync=False,
                                               reason="order ldw before matmuls")
                                if pe_abs is not None:
                                    add_dep_helper(mm.ins, pe_abs.ins, sync=False,
                                                   reason="order absorber before matmuls")

                    # DVE absorber for the self-tick Tile emits on slot WAR
                    if i - 1 >= 0:
                        dve_abs = nc.vector.tensor_copy(
                            touch[:, 2 * i : 2 * i + 2], nege2[:, :2]
                        )
                        add_dep_helper(
                            dve_abs.ins,
                            maxi_insts[i - 1].ins,
                            reason="DVE absorbs hc-slot tick",
                        )

                    hc = hp.tile([P, K], FP32, tag="hc")
                    add_insts[i] = nc.vector.tensor_add(hc[:], hps[:], nege2[:])

                    isl = slice(i * 8, i * 8 + 8)
                    msl = slice((i % 16) * 8, (i % 16) * 8 + 8)
                    nc.vector.max(out=mx_all[:, msl], in_=hc[:])
                    maxi_insts[i] = nc.vector.max_index(
                        out=idx_all[:, isl], in_max=mx_all[:, msl], in_values=hc[:]
                    )
                    # contiguous per-tile winning index (on Pool: keeps the
                    # copy off the DVE critical path)
                    pool_log.append(nc.gpsimd.tensor_copy(
                        idxp_all[:, i : i + 1], idx_all[:, i * 8 : i * 8 + 1]
                    ))

                    if (i + 1) % g == 0:
                        c = i // g
                        rows = slice(c * g * P, (c + 1) * g * P)
                        qt = qt_g[c % 4]
                        if c >= 4:
                            # absorb the qt-tile reuse ticks (whole-tensor
                            # tracked indirect write vs group c-4's readers)
                            ab1 = nc.gpsimd.memset(
                                stouch[:1, (8 * c + 5) % 16 + 32 : (8 * c + 5) % 16 + 33], 0.0
                            )
                            pool_log.append(ab1)
                            add_dep_helper(
                                ab1.ins, qout_insts[c - 4].ins,
                                reason="Pool absorbs qt-reader (qout) tick",
                            )
                        for j in range(g):
                            jj = c * g + j
                            jd = slice(j * DIM, (j + 1) * DIM)
                            swdge(lambda jj=jj, jd=jd: nc.gpsimd.indirect_dma_start(
                                out=qt[:, jd],
                                out_offset=None,
                                in_=e_in[:],
                                in_offset=bass.IndirectOffsetOnAxis(
                                    ap=idxp_all[:, jj : jj + 1], axis=0
                                ),
                            ))
                        # quantize: single-sem producer (the gather)
                        qout_insts[c] = swdge(lambda: nc.gpsimd.dma_start(
                            out=q_out[rows, :].rearrange("(t p) d -> p t d", p=P),
                            in_=qt[:].rearrange("p (t d) -> p t d", t=g),
                        ))
                        # residual: in-place Pool add into the xneg ring,
                        # then one SWDGE out-DMA with a single Pool-sem wait.
                        ngsl = slice(c * g * DIM, (c + 1) * g * DIM)
                        p_abs = None
                        for j in range(g):
                            p_abs = nc.gpsimd.memset(
                                stouch[:1, (8 * c + j) % 16 + 16 : (8 * c + j) % 16 + 17],
                                0.0,
                            )
                            pool_log.append(p_abs)
                            add_dep_helper(
                                p_abs.ins, xneg_dmas[c * g + j].ins,
                                reason="Pool absorbs xneg-load tick",
                            )
                        last_add = None
                        for j in range(g):
                            jsl = slice((c * g + j) * DIM, (c * g + j + 1) * DIM)
                            last_add = nc.gpsimd.tensor_add(
                                xneg_all[:, jsl],
                                qt[:, j * DIM : (j + 1) * DIM],
                                xneg_all[:, jsl],
                            )
                            add_dep_helper(
                                last_add.ins, p_abs.ins, sync=False,
                                reason="order absorber before adds",
                            )
                        pool_log.append(last_add)
                        swdge(lambda: nc.gpsimd.dma_start(
                            out=r_out[rows, :].rearrange("(t p) d -> p t d", p=P),
                            in_=xneg_all[:, ngsl].rearrange(
                                "p (t d) -> p t d", t=g
                            ),
                        ))
                        if (c + 1) % IOUT_GROUPS == 0:
                            i0 = (c + 1 - IOUT_GROUPS) * g
                            i0c = i0
                            ic = i
                            swdge(lambda: nc.gpsimd.dma_start(
                                out=i_out[i0c * P : (ic + 1) * P, :].rearrange(
                                    "(t p) one -> p (t one)", p=P
                                ),
                                in_=idxp_all[:, i0c : ic + 1],
                            ))

                # Pre-drain absorbers: the TileContext tail drain otherwise
                # accumulates one wait per outstanding proc tick (~19), far
                # over walrus's 1-wait limit. A chain of manual drains on the
                # sync engine observes each final tick one at a time.
                finals = (
                    swdge_log[-16:]
                    + hwdge_log[-10:]
                    + pe_log[-1:]
                    + dve_log[-1:]
                    + pool_log[-4:]
                    + [add_insts[n_tiles - 1], maxi_insts[n_tiles - 1]]
                )
                for f in finals:
                    dr = nc.sync.drain()
                    add_dep_helper(dr.ins, f.ins, reason="pre-drain absorber")
    return nc


_NC_CACHE: dict[int, bass.Bass] = {}


def _get_nc(n_tok_core: int = TOK_PER_CORE) -> bass.Bass:
    if n_tok_core not in _NC_CACHE:
        _NC_CACHE[n_tok_core] = build_bass(n_tok_core)
    return _NC_CACHE[n_tok_core]


def make_in_maps(flat: np.ndarray, e: np.ndarray, n_cores: int = N_CORES):
    """Shard + lay out the host-side inputs for each core."""
    n_tok_core = flat.shape[0] // n_cores
    n_tiles = n_tok_core // P
    eT = np.ascontiguousarray(e.T.reshape(2, P, K))
    nege2 = np.broadcast_to(
        -(e.astype(np.float64) ** 2).sum(axis=1).astype(np.float32)[None, :], (P, K)
    )
    nege2 = np.ascontiguousarray(nege2)
    in_maps = []
    for i in range(n_cores):
        shard = flat[i * n_tok_core : (i + 1) * n_tok_core]
        xt2 = np.ascontiguousarray(
            (2.0 * shard).reshape(n_tiles, P, 2, P).transpose(0, 2, 3, 1)
        )
        in_maps.append(
            {
                "xt2": xt2,
                "xneg": np.ascontiguousarray(-shard),
                "embed": np.ascontiguousarray(e),
                "embedT": eT,
                "nege2": nege2,
            }
        )
    return in_maps


def run_device(x: np.ndarray, embed: np.ndarray, trace: bool = False):
    from concourse.bass_utils import run_bass_kernel_spmd

    flat = np.ascontiguousarray(np.asarray(x, dtype=np.float32).reshape(-1, DIM))
    e = np.ascontiguousarray(np.asarray(embed, dtype=np.float32))
    assert flat.shape == (N_TOK, DIM) and e.shape == (K, DIM)

    nc = _get_nc()
    in_maps = make_in_maps(flat, e)
    bres = run_bass_kernel_spmd(nc, in_maps, list(range(N_CORES)), trace=trace)
    res = bres.results
    quantize = np.concatenate([res[i]["quantize"] for i in range(N_CORES)], axis=0)
    embed_ind = (
        np.concatenate([res[i]["embed_ind"] for i in range(N_CORES)], axis=0)
        .reshape(-1)
        .astype(np.int32)
    )
    residual = np.concatenate([res[i]["residual"] for i in range(N_CORES)], axis=0)
    return (quantize, embed_ind, residual), bres


def kernel(x: np.ndarray, embed: np.ndarray):
    outs, _ = run_device(x, embed)
    return outs


# revision 39
# speedup vs baseline: 1.1132x; 1.1132x over previous
"""Trainium2 Bass kernel for EuclideanCodebook (VQ) eval-mode forward.

Full-input contract: kernel(x, embed) -> (quantize, embed_ind, residual)

Data-parallel over 8 NeuronCores (8192 tokens/core), codebook replicated.
Per 128-token tile the device computes scores h = (2x)·e^T - |e|^2 with
fp32 PE matmuls (argmax h == argmin dist, fp32-exact), finds the argmax
with DVE max/max_index, gathers the winning codebook rows with batched
indirect DMAs, and produces the residual as quantize + (-x) via a
DMA-engine accumulate (exact fp32, identical rounding to quantize - x).

The walrus build in this toolchain accepts at most ONE sync wait per
instruction, so the kernel keeps every instruction at <=1 cross wait:
SBUF rings spanning the shard (no slot reuse), real absorber instructions
that pre-advance engine clocks, SWDGE (gpsimd) output DMAs whose producers
collapse to a single semaphore, and a DRAM->DRAM accumulate whose source
is a never-written input tensor (zero producer deps).
"""

import os
import sys

for _p in ("/opt/trn_rl_repo", os.path.expanduser("~/.axon_site/_ro/trn_rl_repo")):
    if os.path.isdir(_p) and _p not in sys.path:
        sys.path.insert(0, _p)

import numpy as np

import concourse.bass as bass
import concourse.mybir as mybir
import concourse.tile as tile
from concourse.tile_rust import add_dep_helper

N_CORES = 8
DIM = 256
K = 1024
N_TOK = 32 * 2048
TOK_PER_CORE = N_TOK // N_CORES
P = 128
FP32 = mybir.dt.float32
U32 = mybir.dt.uint32
BF16 = mybir.dt.bfloat16

GATHER_TILES = 8  # tiles per indirect-DMA gather call
IOUT_GROUPS = 2  # gather groups per embed_ind output DMA


def build_bass(n_tok_core: int = TOK_PER_CORE) -> bass.Bass:
    n_tiles = n_tok_core // P
    g = GATHER_TILES
    assert n_tiles % (g * IOUT_GROUPS) == 0
    nc = bass.Bass()
    xt2_in = nc.declare_dram_parameter("xt2", [n_tiles, 2, P, P], FP32, isOutput=False)
    xneg_in = nc.declare_dram_parameter("xneg", [n_tok_core, DIM], FP32, isOutput=False)
    e_in = nc.declare_dram_parameter("embed", [K, DIM], FP32, isOutput=False)
    et_in = nc.declare_dram_parameter("embedT", [2, P, K], FP32, isOutput=False)
    ne2_in = nc.declare_dram_parameter("nege2", [P, K], FP32, isOutput=False)

    q_out = nc.declare_dram_parameter("quantize", [n_tok_core, DIM], FP32, isOutput=True)
    i_out = nc.declare_dram_parameter("embed_ind", [n_tok_core, 1], U32, isOutput=True)
    r_out = nc.declare_dram_parameter("residual", [n_tok_core, DIM], FP32, isOutput=True)

    with tile.TileContext(nc) as tc:
        with tc.tile_pool(name="const", bufs=1) as cpool:
            eT = [
                cpool.tile([P, K], FP32, tag=f"eT_{h}", name=f"eT_{h}")
                for h in range(2)
            ]
            nege2 = cpool.tile([P, K], FP32, tag="nege2")
            # Rings spanning the whole shard: no slot reuse anywhere.
            xT_all = cpool.tile([P, n_tiles * DIM], FP32, tag="xT_all")
            # one gather-output tile per group: the indirect DMA's write is
            # tracked whole-tensor, so sharing one ring would serialize groups
            qt_g = [
                cpool.tile([P, GATHER_TILES * DIM], FP32, tag=f"qt_{c}",
                           name=f"qt_{c}")
                for c in range(min(4, n_tiles // GATHER_TILES))
            ]
            xneg_all = cpool.tile([P, n_tiles * DIM], FP32, tag="xneg_all")
            idx_all = cpool.tile([P, n_tiles * 8], U32, tag="idx_all")
            idxp_all = cpool.tile([P, n_tiles], U32, tag="idxp_all")
            mx_all = cpool.tile([P, 16 * 8], FP32, tag="mx_all")
            touch = cpool.tile([P, 2 * n_tiles], FP32, tag="touch")
            stouch = cpool.tile([P, 48], FP32, tag="stouch")

            for h in range(2):
                nc.sync.dma_start(out=eT[h][:], in_=et_in[h])
            nc.sync.dma_start(out=nege2[:], in_=ne2_in[:])

            with (
                tc.tile_pool(name="hp", bufs=1) as hp,
                tc.tile_pool(name="psh", bufs=2, space="PSUM") as psh,
            ):
                # absorb const-load ticks on their consumer engines
                for h in range(2):
                    nc.tensor.ldweights(weights=eT[h][:, :8].bitcast(BF16))
                nc.vector.tensor_copy(touch[:, :2], nege2[:, :2])

                add_insts = {}
                maxi_insts = {}
                qout_insts = {}
                swdge_log = []
                hwdge_log = []
                pe_log = []
                dve_log = []
                pool_log = []

                def swdge(emit_fn):
                    # SWDGE sem-slot reuse (lap >= 2 of the 8-sem round robin)
                    # makes Tile add a ring-credit wait; pre-absorb it on a
                    # tiny Pool instruction so the DMA keeps <=1 wait.
                    n = len(swdge_log)
                    for k in range(max(0, n - 8), n):
                        ab = nc.gpsimd.memset(
                            stouch[:1, k % 16 : k % 16 + 1], 0.0
                        )
                        pool_log.append(ab)
                        add_dep_helper(
                            ab.ins, swdge_log[k].ins,
                            reason="Pool absorbs SWDGE ring tick",
                        )
                    inst = emit_fn()
                    swdge_log.append(inst)
                    return inst

                def logged(log, inst):
                    log.append(inst)
                    return inst

                xneg_dmas = {}
                for i in range(n_tiles):
                    csl = slice(i * DIM, (i + 1) * DIM)

                    hwdge_log.append(nc.sync.dma_start(
                        out=xT_all[:, csl].rearrange("p (h t) -> p h t", h=2),
                        in_=xt2_in[i].rearrange("h d t -> d h t"),
                    ))
                    xneg_dmas[i] = nc.sync.dma_start(
                        out=xneg_all[:, csl], in_=xneg_in[i * P : (i + 1) * P, :]
                    )
                    hwdge_log.append(xneg_dmas[i])
                    # PE absorbers: the xT DMA tick, then the DVE tick that
                    # freed this tile's PSUM slot.
                    ldw = nc.tensor.ldweights(
                        weights=xT_all[:, i * DIM : i * DIM + 8].bitcast(BF16)
                    )
                    pe_abs = None
                    if i - 2 >= 0:
                        pe_abs = nc.tensor.ldweights(weights=eT[0][:, :8].bitcast(BF16))
                        add_dep_helper(
                            pe_abs.ins,
                            add_insts[i - 2].ins,
                            reason="PE absorbs PSUM-release tick",
                        )

                    hps = psh.tile([P, K], FP32, tag="hps", space="PSUM")
                    first = True
                    for half in range(2):
                        sl = slice(half * 512, (half + 1) * 512)
                        for h in range(2):
                            mm = nc.tensor.matmul(
                                hps[:, sl],
                                lhsT=xT_all[:, i * DIM + h * P : i * DIM + (h + 1) * P],
                                rhs=eT[h][:, sl],
                                start=(h == 0),
                                stop=(h == 1),
                            )
                            pe_log.append(mm)
                            if first:
                                first = False
                                add_dep_helper(mm.ins, ldw.ins, sync=False,
                                               reason="order ldw before matmuls")
                                if pe_abs is not None:
                                    add_dep_helper(mm.ins, pe_abs.ins, sync=False,
                                                   reason="order absorber before matmuls")

                    # DVE absorber for the self-tick Tile emits on slot WAR
                    if i - 1 >= 0:
                        dve_abs = nc.vector.tensor_copy(
                            touch[:, 2 * i : 2 * i + 2], nege2[:, :2]
                        )
                        add_dep_helper(
                            dve_abs.ins,
                            maxi_insts[i - 1].ins,
                            reason="DVE absorbs hc-slot tick",
                        )

                    hc = hp.tile([P, K], FP32, tag="hc")
                    add_insts[i] = nc.vector.tensor_add(hc[:], hps[:], nege2[:])

                    isl = slice(i * 8, i * 8 + 8)
                    msl = slice((i % 16) * 8, (i % 16) * 8 + 8)
                    nc.vector.max(out=mx_all[:, msl], in_=hc[:])
                    maxi_insts[i] = nc.vector.max_index(
                        out=idx_all[:, isl], in_max=mx_all[:, msl], in_values=hc[:]
                    )
                    # contiguous per-tile winning index
                    dve_log.append(nc.vector.tensor_copy(
                        idxp_all[:, i : i + 1], idx_all[:, i * 8 : i * 8 + 1]
                    ))

                    if (i + 1) % g == 0:
                        c = i // g
                        rows = slice(c * g * P, (c + 1) * g * P)
                        qt = qt_g[c % 4]
                        if c >= 4:
                            # absorb the qt-tile reuse ticks (whole-tensor
                            # tracked indirect write vs group c-4's readers)
                            ab1 = nc.gpsimd.memset(
                                stouch[:1, (8 * c + 5) % 16 + 32 : (8 * c + 5) % 16 + 33], 0.0
                            )
                            pool_log.append(ab1)
                            add_dep_helper(
                                ab1.ins, qout_insts[c - 4].ins,
                                reason="Pool absorbs qt-reader (qout) tick",
                            )
                        for j in range(g):
                            jj = c * g + j
                            jd = slice(j * DIM, (j + 1) * DIM)
                            swdge(lambda jj=jj, jd=jd: nc.gpsimd.indirect_dma_start(
                                out=qt[:, jd],
                                out_offset=None,
                                in_=e_in[:],
                                in_offset=bass.IndirectOffsetOnAxis(
                                    ap=idxp_all[:, jj : jj + 1], axis=0
                                ),
                            ))
                        # quantize: single-sem producer (the gather)
                        qout_insts[c] = swdge(lambda: nc.gpsimd.dma_start(
                            out=q_out[rows, :].rearrange("(t p) d -> p t d", p=P),
                            in_=qt[:].rearrange("p (t d) -> p t d", t=g),
                        ))
                        # residual: in-place Pool add into the xneg ring,
                        # then one SWDGE out-DMA with a single Pool-sem wait.
                        ngsl = slice(c * g * DIM, (c + 1) * g * DIM)
                        p_abs = None
                        for j in range(g):
                            p_abs = nc.gpsimd.memset(
                                stouch[:1, (8 * c + j) % 16 + 16 : (8 * c + j) % 16 + 17],
                                0.0,
                            )
                            pool_log.append(p_abs)
                            add_dep_helper(
                                p_abs.ins, xneg_dmas[c * g + j].ins,
                                reason="Pool absorbs xneg-load tick",
                            )
                        last_add = None
                        for j in range(g):
                            jsl = slice((c * g + j) * DIM, (c * g + j + 1) * DIM)
                            last_add = nc.gpsimd.tensor_add(
                                xneg_all[:, jsl],
                                qt[:, j * DIM : (j + 1) * DIM],
                                xneg_all[:, jsl],
                            )
                            add_dep_helper(
                                last_add.ins, p_abs.ins, sync=False,
                                reason="order absorber before adds",
                            )
                        pool_log.append(last_add)
                        swdge(lambda: nc.gpsimd.dma_start(
                            out=r_out[rows, :].rearrange("(t p) d -> p t d", p=P),
                            in_=xneg_all[:, ngsl].rearrange(
                                "p (t d) -> p t d", t=g
                            ),
                        ))
                        if (c + 1) % IOUT_GROUPS == 0:
                            i0 = (c + 1 - IOUT_GROUPS) * g
                            i0c = i0
                            ic = i
                            swdge(lambda: nc.gpsimd.dma_start(
                                out=i_out[i0c * P : (ic + 1) * P, :].rearrange(
                                    "(t p) one -> p (t one)", p=P
                                ),
                                in_=idxp_all[:, i0c : ic + 1],
                            ))

                # Pre-drain absorbers: the TileContext tail drain otherwise
                # accumulates one wait per outstanding proc tick (~19), far
                # over walrus's 1-wait limit. A chain of manual drains on the
                # sync engine observes each final tick one at a time.
                finals = (
                    swdge_log[-16:]
                    + hwdge_log[-10:]
                    + pe_log[-1:]
                    + dve_log[-1:]
                    + pool_log[-4:]
                    + [add_insts[n_tiles - 1], maxi_insts[n_tiles - 1]]
                )
                for f in finals:
                    dr = nc.sync.drain()
                    add_dep_helper(dr.ins, f.ins, reason="pre-drain absorber")
    return nc


_NC_CACHE: dict[int, bass.Bass] = {}


def _get_nc(n_tok_core: int = TOK_PER_CORE) -> bass.Bass:
    if n_tok_core not in _NC_CACHE:
        _NC_CACHE[n_tok_core] = build_bass(n_tok_core)
    return _NC_CACHE[n_tok_core]


def make_in_maps(flat: np.ndarray, e: np.ndarray, n_cores: int = N_CORES):
    """Shard + lay out the host-side inputs for each core."""
    n_tok_core = flat.shape[0] // n_cores
    n_tiles = n_tok_core // P
    eT = np.ascontiguousarray(e.T.reshape(2, P, K))
    nege2 = np.broadcast_to(
        -(e.astype(np.float64) ** 2).sum(axis=1).astype(np.float32)[None, :], (P, K)
    )
    nege2 = np.ascontiguousarray(nege2)
    in_maps = []
    for i in range(n_cores):
        shard = flat[i * n_tok_core : (i + 1) * n_tok_core]
        xt2 = np.ascontiguousarray(
            (2.0 * shard).reshape(n_tiles, P, 2, P).transpose(0, 2, 3, 1)
        )
        in_maps.append(
            {
                "xt2": xt2,
                "xneg": np.ascontiguousarray(-shard),
                "embed": np.ascontiguousarray(e),
                "embedT": eT,
                "nege2": nege2,
            }
        )
    return in_maps


def run_device(x: np.ndarray, embed: np.ndarray, trace: bool = False):
    from concourse.bass_utils import run_bass_kernel_spmd

    flat = np.ascontiguousarray(np.asarray(x, dtype=np.float32).reshape(-1, DIM))
    e = np.ascontiguousarray(np.asarray(embed, dtype=np.float32))
    assert flat.shape == (N_TOK, DIM) and e.shape == (K, DIM)

    nc = _get_nc()
    in_maps = make_in_maps(flat, e)
    bres = run_bass_kernel_spmd(nc, in_maps, list(range(N_CORES)), trace=trace)
    res = bres.results
    quantize = np.concatenate([res[i]["quantize"] for i in range(N_CORES)], axis=0)
    embed_ind = (
        np.concatenate([res[i]["embed_ind"] for i in range(N_CORES)], axis=0)
        .reshape(-1)
        .astype(np.int32)
    )
    residual = np.concatenate([res[i]["residual"] for i in range(N_CORES)], axis=0)
    return (quantize, embed_ind, residual), bres


def kernel(x: np.ndarray, embed: np.ndarray):
    outs, _ = run_device(x, embed)
    return outs


# revision 40
# speedup vs baseline: 1.1217x; 1.0076x over previous
"""Trainium2 Bass kernel for EuclideanCodebook (VQ) eval-mode forward.

Full-input contract: kernel(x, embed) -> (quantize, embed_ind, residual)

Data-parallel over 8 NeuronCores (8192 tokens/core), codebook replicated.
Per 128-token tile the device computes scores h = (2x)·e^T - |e|^2 with
fp32 PE matmuls (argmax h == argmin dist, fp32-exact), finds the argmax
with DVE max/max_index, gathers the winning codebook rows with batched
indirect DMAs, and produces the residual as quantize + (-x) via a
DMA-engine accumulate (exact fp32, identical rounding to quantize - x).

The walrus build in this toolchain accepts at most ONE sync wait per
instruction, so the kernel keeps every instruction at <=1 cross wait:
SBUF rings spanning the shard (no slot reuse), real absorber instructions
that pre-advance engine clocks, SWDGE (gpsimd) output DMAs whose producers
collapse to a single semaphore, and a DRAM->DRAM accumulate whose source
is a never-written input tensor (zero producer deps).
"""

import os
import sys

for _p in ("/opt/trn_rl_repo", os.path.expanduser("~/.axon_site/_ro/trn_rl_repo")):
    if os.path.isdir(_p) and _p not in sys.path:
        sys.path.insert(0, _p)

import numpy as np

import concourse.bass as bass
import concourse.mybir as mybir
import concourse.tile as tile
from concourse.tile_rust import add_dep_helper

N_CORES = 8
DIM = 256
K = 1024
N_TOK = 32 * 2048
TOK_PER_CORE = N_TOK // N_CORES
P = 128
FP32 = mybir.dt.float32
U32 = mybir.dt.uint32
BF16 = mybir.dt.bfloat16

GATHER_TILES = 8  # tiles per indirect-DMA gather call
IOUT_GROUPS = 2  # gather groups per embed_ind output DMA


def build_bass(n_tok_core: int = TOK_PER_CORE) -> bass.Bass:
    n_tiles = n_tok_core // P
    g = GATHER_TILES
    assert n_tiles % (g * IOUT_GROUPS) == 0
    nc = bass.Bass()
    xt2_in = nc.declare_dram_parameter("xt2", [n_tiles, 2, P, P], FP32, isOutput=False)
    xneg_in = nc.declare_dram_parameter("xneg", [n_tok_core, DIM], FP32, isOutput=False)
    e_in = nc.declare_dram_parameter("embed", [K, DIM], FP32, isOutput=False)
    et_in = nc.declare_dram_parameter("embedT", [2, P, K], FP32, isOutput=False)
    ne2_in = nc.declare_dram_parameter("nege2", [P, K], FP32, isOutput=False)

    q_out = nc.declare_dram_parameter("quantize", [n_tok_core, DIM], FP32, isOutput=True)
    i_out = nc.declare_dram_parameter("embed_ind", [n_tok_core, 1], U32, isOutput=True)
    r_out = nc.declare_dram_parameter("residual", [n_tok_core, DIM], FP32, isOutput=True)

    with tile.TileContext(nc) as tc:
        with tc.tile_pool(name="const", bufs=1) as cpool:
            eT = [
                cpool.tile([P, K], FP32, tag=f"eT_{h}", name=f"eT_{h}")
                for h in range(2)
            ]
            nege2 = cpool.tile([P, K], FP32, tag="nege2")
            # Rings spanning the whole shard: no slot reuse anywhere.
            xT_all = cpool.tile([P, n_tiles * DIM], FP32, tag="xT_all")
            # one gather-output tile per group: the indirect DMA's write is
            # tracked whole-tensor, so sharing one ring would serialize groups
            qt_g = [
                cpool.tile([P, GATHER_TILES * DIM], FP32, tag=f"qt_{c}",
                           name=f"qt_{c}")
                for c in range(min(4, n_tiles // GATHER_TILES))
            ]
            xneg_all = cpool.tile([P, n_tiles * DIM], FP32, tag="xneg_all")
            idx_all = cpool.tile([P, n_tiles * 8], U32, tag="idx_all")
            idxp_all = cpool.tile([P, n_tiles], U32, tag="idxp_all")
            mx_all = cpool.tile([P, 16 * 8], FP32, tag="mx_all")
            touch = cpool.tile([P, 2 * n_tiles], FP32, tag="touch")
            stouch = cpool.tile([P, 48], FP32, tag="stouch")

            for h in range(2):
                nc.sync.dma_start(out=eT[h][:], in_=et_in[h])
            nc.sync.dma_start(out=nege2[:], in_=ne2_in[:])

            with (
                tc.tile_pool(name="hp", bufs=1) as hp,
                tc.tile_pool(name="psh", bufs=3, space="PSUM") as psh,
            ):
                # absorb const-load ticks on their consumer engines
                for h in range(2):
                    nc.tensor.ldweights(weights=eT[h][:, :8].bitcast(BF16))
                nc.vector.tensor_copy(touch[:, :2], nege2[:, :2])

                add_insts = {}
                maxi_insts = {}
                qout_insts = {}
                swdge_log = []
                hwdge_log = []
                pe_log = []
                dve_log = []
                pool_log = []

                def swdge(emit_fn):
                    # SWDGE sem-slot reuse (lap >= 2 of the 8-sem round robin)
                    # makes Tile add a ring-credit wait; pre-absorb it on a
                    # tiny Pool instruction so the DMA keeps <=1 wait.
                    n = len(swdge_log)
                    for k in range(max(0, n - 8), n):
                        ab = nc.gpsimd.memset(
                            stouch[:1, k % 16 : k % 16 + 1], 0.0
                        )
                        pool_log.append(ab)
                        add_dep_helper(
                            ab.ins, swdge_log[k].ins,
                            reason="Pool absorbs SWDGE ring tick",
                        )
                    inst = emit_fn()
                    swdge_log.append(inst)
                    return inst

                def logged(log, inst):
                    log.append(inst)
                    return inst

                xneg_dmas = {}
                for i in range(n_tiles):
                    csl = slice(i * DIM, (i + 1) * DIM)

                    hwdge_log.append(nc.sync.dma_start(
                        out=xT_all[:, csl].rearrange("p (h t) -> p h t", h=2),
                        in_=xt2_in[i].rearrange("h d t -> d h t"),
                    ))
                    xneg_dmas[i] = nc.sync.dma_start(
                        out=xneg_all[:, csl], in_=xneg_in[i * P : (i + 1) * P, :]
                    )
                    hwdge_log.append(xneg_dmas[i])
                    # PE absorbers: the xT DMA tick, then the DVE tick that
                    # freed this tile's PSUM slot.
                    ldw = nc.tensor.ldweights(
                        weights=xT_all[:, i * DIM : i * DIM + 8].bitcast(BF16)
                    )
                    pe_abs = None
                    if i - 3 >= 0:
                        pe_abs = nc.tensor.ldweights(weights=eT[0][:, :8].bitcast(BF16))
                        add_dep_helper(
                            pe_abs.ins,
                            add_insts[i - 3].ins,
                            reason="PE absorbs PSUM-release tick",
                        )

                    hps = psh.tile([P, K], FP32, tag="hps", space="PSUM")
                    first = True
                    for half in range(2):
                        sl = slice(half * 512, (half + 1) * 512)
                        for h in range(2):
                            mm = nc.tensor.matmul(
                                hps[:, sl],
                                lhsT=xT_all[:, i * DIM + h * P : i * DIM + (h + 1) * P],
                                rhs=eT[h][:, sl],
                                start=(h == 0),
                                stop=(h == 1),
                            )
                            pe_log.append(mm)
                            if first:
                                first = False
                                add_dep_helper(mm.ins, ldw.ins, sync=False,
                                               reason="order ldw before matmuls")
                                if pe_abs is not None:
                                    add_dep_helper(mm.ins, pe_abs.ins, sync=False,
                                                   reason="order absorber before matmuls")

                    # DVE absorber for the self-tick Tile emits on slot WAR
                    if i - 1 >= 0:
                        dve_abs = nc.vector.tensor_copy(
                            touch[:, 2 * i : 2 * i + 2], nege2[:, :2]
                        )
                        add_dep_helper(
                            dve_abs.ins,
                            maxi_insts[i - 1].ins,
                            reason="DVE absorbs hc-slot tick",
                        )

                    hc = hp.tile([P, K], FP32, tag="hc")
                    add_insts[i] = nc.vector.tensor_add(hc[:], hps[:], nege2[:])

                    isl = slice(i * 8, i * 8 + 8)
                    msl = slice((i % 16) * 8, (i % 16) * 8 + 8)
                    nc.vector.max(out=mx_all[:, msl], in_=hc[:])
                    maxi_insts[i] = nc.vector.max_index(
                        out=idx_all[:, isl], in_max=mx_all[:, msl], in_values=hc[:]
                    )
                    # contiguous per-tile winning index
                    dve_log.append(nc.vector.tensor_copy(
                        idxp_all[:, i : i + 1], idx_all[:, i * 8 : i * 8 + 1]
                    ))

                    if (i + 1) % g == 0:
                        c = i // g
                        rows = slice(c * g * P, (c + 1) * g * P)
                        qt = qt_g[c % 4]
                        if c >= 4:
                            # absorb the qt-tile reuse ticks (whole-tensor
                            # tracked indirect write vs group c-4's readers)
                            ab1 = nc.gpsimd.memset(
                                stouch[:1, (8 * c + 5) % 16 + 32 : (8 * c + 5) % 16 + 33], 0.0
                            )
                            pool_log.append(ab1)
                            add_dep_helper(
                                ab1.ins, qout_insts[c - 4].ins,
                                reason="Pool absorbs qt-reader (qout) tick",
                            )
                        for j in range(g):
                            jj = c * g + j
                            jd = slice(j * DIM, (j + 1) * DIM)
                            swdge(lambda jj=jj, jd=jd: nc.gpsimd.indirect_dma_start(
                                out=qt[:, jd],
                                out_offset=None,
                                in_=e_in[:],
                                in_offset=bass.IndirectOffsetOnAxis(
                                    ap=idxp_all[:, jj : jj + 1], axis=0
                                ),
                            ))
                        # quantize: single-sem producer (the gather)
                        qout_insts[c] = swdge(lambda: nc.gpsimd.dma_start(
                            out=q_out[rows, :].rearrange("(t p) d -> p t d", p=P),
                            in_=qt[:].rearrange("p (t d) -> p t d", t=g),
                        ))
                        # residual: in-place Pool add into the xneg ring,
                        # then one SWDGE out-DMA with a single Pool-sem wait.
                        ngsl = slice(c * g * DIM, (c + 1) * g * DIM)
                        p_abs = None
                        for j in range(g):
                            p_abs = nc.gpsimd.memset(
                                stouch[:1, (8 * c + j) % 16 + 16 : (8 * c + j) % 16 + 17],
                                0.0,
                            )
                            pool_log.append(p_abs)
                            add_dep_helper(
                                p_abs.ins, xneg_dmas[c * g + j].ins,
                                reason="Pool absorbs xneg-load tick",
                            )
                        last_add = None
                        for j in range(g):
                            jsl = slice((c * g + j) * DIM, (c * g + j + 1) * DIM)
                            last_add = nc.gpsimd.tensor_add(
                                xneg_all[:, jsl],
                                qt[:, j * DIM : (j + 1) * DIM],
                                xneg_all[:, jsl],
                            )
                            add_dep_helper(
                                last_add.ins, p_abs.ins, sync=False,
                                reason="order absorber before adds",
                            )
                        pool_log.append(last_add)
                        swdge(lambda: nc.gpsimd.dma_start(
                            out=r_out[rows, :].rearrange("(t p) d -> p t d", p=P),
                            in_=xneg_all[:, ngsl].rearrange(
                                "p (t d) -> p t d", t=g
                            ),
                        ))
                        if (c + 1) % IOUT_GROUPS == 0:
                            i0 = (c + 1 - IOUT_GROUPS) * g
                            i0c = i0
                            ic = i
                            swdge(lambda: nc.gpsimd.dma_start(
                                out=i_out[i0c * P : (ic + 1) * P, :].rearrange(
                                    "(t p) one -> p (t one)", p=P
                                ),
                                in_=idxp_all[:, i0c : ic + 1],
                            ))

                # Pre-drain absorbers: the TileContext tail drain otherwise
                # accumulates one wait per outstanding proc tick (~19), far
                # over walrus's 1-wait limit. A chain of manual drains on the
                # sync engine observes each final tick one at a time.
                finals = (
                    swdge_log[-16:]
                    + hwdge_log[-10:]
                    + pe_log[-1:]
                    + dve_log[-1:]
                    + pool_log[-4:]
                    + [add_insts[n_tiles - 1], maxi_insts[n_tiles - 1]]
                )
                for f in finals:
                    dr = nc.sync.drain()
                    add_dep_helper(dr.ins, f.ins, reason="pre-drain absorber")
    return nc


_NC_CACHE: dict[int, bass.Bass] = {}


def _get_nc(n_tok_core: int = TOK_PER_CORE) -> bass.Bass:
    if n_tok_core not in _NC_CACHE:
        _NC_CACHE[n_tok_core] = build_bass(n_tok_core)
    return _NC_CACHE[n_tok_core]


def make_in_maps(flat: np.ndarray, e: np.ndarray, n_cores: int = N_CORES):
    """Shard + lay out the host-side inputs for each core."""
    n_tok_core = flat.shape[0] // n_cores
    n_tiles = n_tok_core // P
    eT = np.ascontiguousarray(e.T.reshape(2, P, K))
    nege2 = np.broadcast_to(
        -(e.astype(np.float64) ** 2).sum(axis=1).astype(np.float32)[None, :], (P, K)
    )
    nege2 = np.ascontiguousarray(nege2)
    in_maps = []
    for i in range(n_cores):
        shard = flat[i * n_tok_core : (i + 1) * n_tok_core]
        xt2 = np.ascontiguousarray(
            (2.0 * shard).reshape(n_tiles, P, 2, P).transpose(0, 2, 3, 1)
        )
        in_maps.append(
            {
                "xt2": xt2,
                "xneg": np.ascontiguousarray(-shard),
                "embed": np.ascontiguousarray(e),
                "embedT": eT,
                "nege2": nege2,
            }
        )
    return in_maps


def run_device(x: np.ndarray, embed: np.ndarray, trace: bool = False):
    from concourse.bass_utils import run_bass_kernel_spmd

    flat = np.ascontiguousarray(np.asarray(x, dtype=np.float32).reshape(-1, DIM))
    e = np.ascontiguousarray(np.asarray(embed, dtype=np.float32))
    assert flat.shape == (N_TOK, DIM) and e.shape == (K, DIM)

    nc = _get_nc()
    in_maps = make_in_maps(flat, e)
    bres = run_bass_kernel_spmd(nc, in_maps, list(range(N_CORES)), trace=trace)
    res = bres.results
    quantize = np.concatenate([res[i]["quantize"] for i in range(N_CORES)], axis=0)
    embed_ind = (
        np.concatenate([res[i]["embed_ind"] for i in range(N_CORES)], axis=0)
        .reshape(-1)
        .astype(np.int32)
    )
    residual = np.concatenate([res[i]["residual"] for i in range(N_CORES)], axis=0)
    return (quantize, embed_ind, residual), bres


def kernel(x: np.ndarray, embed: np.ndarray):
    outs, _ = run_device(x, embed)
    return outs
